# revision 15
# baseline (speedup 1.0000x reference)
"""Trainium2 Bass kernel for nn_CysInteractLayer (GNN message-passing layer).

out = BN(lrelu(lrelu(lrelu([ef | nf[src]+nf[dst]] @ W1 + b1) @ W2 + b2) @ W3 + b3))

Sharding: edges across 8 NeuronCores; node_feats/weights replicated
(as per-half local node tables so dma_gather's int16 indices suffice);
BN batch stats all-reduced across cores on-device.  Final output stays
feature-major bf16 [128, E/8] and is transposed/upcast on host.

Gather strategy: NON-transpose dma_gather calls round-robined across the
4 SWDGE queues (each queue runs on its own Q7 core pair, so 4 calls'
descriptor generation overlaps; the transpose-mode gather is NOT safe to
run concurrently across queues - its XBAR spray streams interleave and
corrupt data).  Gathered rows land edge-major [128, rows, 128]; src+dst
are summed on DVE and transposed to feature-major on the PE (via
identity matmul into PSUM), then the 3-layer MLP runs as before.
"""
import numpy as np
import ml_dtypes

import concourse.bass as bass
import concourse.bacc as bacc
import concourse.tile as tile
from concourse import mybir
from concourse.bass_utils import run_bass_kernel_spmd
from concourse.masks import make_identity

F32 = mybir.dt.float32
BF16 = mybir.dt.bfloat16
I16 = mybir.dt.int16

# problem constants (hardcoded per harness contract)
V, E = 50000, 400000
ND, ED, OUT = 128, 64, 128
IN = ND + ED
NEG_SLOPE = 0.01
BN_EPS = 1e-5

NCORES = 8
ES = E // NCORES            # 50000 edges per core

class Cfg:
    """Geometry of the per-core kernel."""

    def __init__(self, es=ES, halves=2, calls_per_half=10, blocks_per_call=5,
                 tn=32768, e_total=E, skip_collective=False, nqueues=4,
                 gsd_bufs=6):
        self.skip_collective = skip_collective
        self.NQ = nqueues                   # SWDGE queues for gather round-robin
        self.GSD_BUFS = gsd_bufs
        self.ES = es                        # valid edges per core
        self.H = halves
        self.C = calls_per_half
        self.B = blocks_per_call
        self.BLK = 512
        self.TN = tn                        # local table rows (padded)
        self.E_TOTAL = e_total
        self.HP = self.C * self.B * self.BLK   # padded edges per half
        self.EP = self.H * self.HP             # padded edges per core
        self.HV = es // halves                 # valid edges per half
        assert es % halves == 0
        assert self.HP >= self.HV
        self.NB = self.H * self.C * self.B     # total blocks
        self.NCALL = self.B * self.BLK         # edges per gather call
        self.R = self.NCALL // 128             # row-chunks per call (per src/dst)
        self.WCOLS = self.NCALL // 16          # wrapped idx cols per call

    def block_valid(self, b):
        """Valid columns in global block b (rest is padding)."""
        h, lb = divmod(b, self.C * self.B)
        lo = lb * self.BLK
        return int(np.clip(self.HV - lo, 0, self.BLK))


CFG = Cfg()

_PROG_CACHE = {}


def build_program(cfg):
    nc = bacc.Bacc(None, target_bir_lowering=False, num_swdge_queues=4)

    tabs = [nc.declare_dram_parameter(f"tab{h}", [cfg.TN, ND], BF16, isOutput=False)
            for h in range(cfg.H)]
    srcw = nc.declare_dram_parameter("srcw", [128, 2 * cfg.H * cfg.C * cfg.WCOLS], I16, isOutput=False)
    eft = nc.declare_dram_parameter("eft", [ED, cfg.EP], BF16, isOutput=False)
    w1e = nc.declare_dram_parameter("w1e", [ED, OUT], BF16, isOutput=False)
    w1m = nc.declare_dram_parameter("w1m", [ND, OUT], BF16, isOutput=False)
    w2 = nc.declare_dram_parameter("w2", [OUT, OUT], BF16, isOutput=False)
    w3 = nc.declare_dram_parameter("w3", [OUT, OUT], BF16, isOutput=False)
    bias = nc.declare_dram_parameter("bias", [128, 3], F32, isOutput=False)   # b1|b2|b3
    gb = nc.declare_dram_parameter("gb", [128, 2], F32, isOutput=False)       # gamma|beta
    out = nc.declare_dram_parameter("out", [128, cfg.EP], BF16, isOutput=True)

    LR = mybir.ActivationFunctionType.Lrelu
    IDENT = mybir.ActivationFunctionType.Identity
    BLK = cfg.BLK
    CPB = BLK // 128          # row-chunks per block (4)

    with tile.TileContext(nc) as tc:
        with (
            tc.tile_pool(name="singles", bufs=1) as singles,
            tc.tile_pool(name="hres", bufs=1) as hres,
            tc.tile_pool(name="eftp", bufs=2) as eftp,
            tc.tile_pool(name="swp", bufs=cfg.GSD_BUFS + 2) as swp,
            tc.tile_pool(name="gsdp", bufs=cfg.GSD_BUFS) as gsdp,
            tc.tile_pool(name="sums", bufs=3) as sums,
            tc.tile_pool(name="gtp", bufs=3) as gtp,
            tc.tile_pool(name="work", bufs=3) as work,
            tc.tile_pool(name="ph2", bufs=4) as ph2,
            tc.tile_pool(name="zps", bufs=4, space="PSUM") as zps,
            tc.tile_pool(name="zpt", bufs=2, space="PSUM") as zpt,
            tc.tile_pool(name="dram", bufs=1, space="DRAM") as dram,
        ):
            # ---- static state ------------------------------------------------
            w1e_t = singles.tile([ED, OUT], BF16)
            nc.sync.dma_start(out=w1e_t[:], in_=w1e[:, :])
            w1m_t = singles.tile([ND, OUT], BF16)
            nc.sync.dma_start(out=w1m_t[:], in_=w1m[:, :])
            w2_t = singles.tile([OUT, OUT], BF16)
            nc.sync.dma_start(out=w2_t[:], in_=w2[:, :])
            w3_t = singles.tile([OUT, OUT], BF16)
            nc.sync.dma_start(out=w3_t[:], in_=w3[:, :])
            bias_t = singles.tile([128, 3], F32)
            nc.sync.dma_start(out=bias_t[:], in_=bias[:, :])
            gb_t = singles.tile([128, 2], F32)
            nc.sync.dma_start(out=gb_t[:], in_=gb[:, :])
            ident_t = singles.tile([128, 128], BF16)
            make_identity(nc, ident_t[:])

            h3res = hres.tile([128, cfg.EP], BF16)
            sum_stripe = singles.tile([128, cfg.NB], F32)
            sq_stripe = singles.tile([128, cfg.NB], F32)
            # fully-padded blocks are skipped in the loop below; zero the
            # stripes so their (never-written) columns don't poison the stats
            nc.vector.memset(sum_stripe[:], 0.0)
            nc.vector.memset(sq_stripe[:], 0.0)

            # ---- phase 1: gather + sum + transpose + MLP + stats -------------
            for ci in range(cfg.H * cfg.C):
                h = ci // cfg.C
                call_e0 = ci * cfg.NCALL
                eft_t = eftp.tile([ED, cfg.NCALL], BF16, tag="eft")
                nc.sync.dma_start(out=eft_t[:], in_=eft[:, call_e0:call_e0 + cfg.NCALL])

                sw_t = swp.tile([128, 2 * cfg.WCOLS], I16, tag="sw")
                nc.sync.dma_start(out=sw_t[:],
                                  in_=srcw[:, ci * 2 * cfg.WCOLS:(ci + 1) * 2 * cfg.WCOLS])
                # non-transpose gather: src rows land in chunks [0,R), dst rows
                # in chunks [R,2R); edge (c*128+p) of this call -> [p, c, :]
                gsd = gsdp.tile([128, 2 * cfg.R, ND], BF16, tag="gsd")
                nc.gpsimd.dma_gather(
                    out_ap=gsd[:, :, :], in_ap=tabs[h][:, :],
                    idxs_ap=sw_t[:, :],
                    num_idxs=2 * cfg.NCALL, num_idxs_reg=2 * cfg.NCALL,
                    elem_size=ND, transpose=False, single_packet=False,
                    queue_num=ci % cfg.NQ,
                )

                for b in range(cfg.B):
                    gb_idx = ci * cfg.B + b
                    vb = cfg.block_valid(gb_idx)
                    if vb == 0:
                        continue
                    co = b * BLK              # col offset within call
                    e0 = call_e0 + co         # global (padded) edge offset
                    c0 = b * CPB              # first row-chunk of this block

                    # m = nf[src] + nf[dst], edge-major [128, BLK]
                    s_t = sums.tile([128, CPB, 128], BF16, tag="s")
                    nc.vector.tensor_add(
                        out=s_t[:, :, :],
                        in0=gsd[:, c0:c0 + CPB, :],
                        in1=gsd[:, cfg.R + c0:cfg.R + c0 + CPB, :])
                    # transpose to feature-major via PE identity matmuls
                    zt = zpt.tile([128, BLK], BF16, tag="zt")
                    for c in range(CPB):
                        nc.tensor.transpose(zt[:, c * 128:(c + 1) * 128],
                                            s_t[:, c, :], ident_t[:])
                    gt = gtp.tile([128, BLK], BF16, tag="gt")
                    nc.vector.tensor_copy(out=gt[:], in_=zt[:])

                    zp = zps.tile([128, BLK], F32, tag="z")
                    nc.tensor.matmul(zp[:, :], lhsT=w1m_t[:], rhs=gt[:],
                                     start=True, stop=False)
                    nc.tensor.matmul(zp[:, :], lhsT=w1e_t[:], rhs=eft_t[:, co:co + BLK],
                                     start=False, stop=True)
                    h1 = work.tile([128, BLK], BF16, tag="h1")
                    nc.scalar.activation(out=h1[:], in_=zp[:], func=LR,
                                         bias=bias_t[:, 0:1], scale=1.0, alpha=NEG_SLOPE)
                    zp2 = zps.tile([128, BLK], F32, tag="z")
                    nc.tensor.matmul(zp2[:, :], lhsT=w2_t[:], rhs=h1[:], start=True, stop=True)
                    h2 = work.tile([128, BLK], BF16, tag="h2")
                    nc.scalar.activation(out=h2[:], in_=zp2[:], func=LR,
                                         bias=bias_t[:, 1:2], scale=1.0, alpha=NEG_SLOPE)
                    zp3 = zps.tile([128, BLK], F32, tag="z")
                    nc.tensor.matmul(zp3[:, :], lhsT=w3_t[:], rhs=h2[:], start=True, stop=True)
                    h3 = h3res[:, e0:e0 + BLK]
                    nc.scalar.activation(out=h3[:, :vb], in_=zp3[:, :vb], func=LR,
                                         bias=bias_t[:, 2:3], scale=1.0, alpha=NEG_SLOPE,
                                         accum_out=sum_stripe[:, gb_idx:gb_idx + 1])
                    if vb < BLK:
                        nc.vector.memset(h3[:, vb:], 0.0)
                    sq_scr = work.tile([128, BLK], BF16, tag="sq")
                    nc.vector.tensor_mul(out=sq_scr[:], in0=h3[:, :], in1=h3[:, :])
                    nc.vector.tensor_reduce(
                        out=sq_stripe[:, gb_idx:gb_idx + 1], in_=sq_scr[:],
                        axis=mybir.AxisListType.X, op=mybir.AluOpType.add)

                if ci == cfg.H * cfg.C - 4:
                    # early barrier: dummy allreduce on the CC cores, hidden
                    # under the remaining gather calls.  Absorbs inter-core
                    # launch stagger so the real stats allreduce at the end
                    # waits only on post-barrier jitter.
                    bar_in = dram.tile([128, 1], F32)
                    bar_out = dram.tile([128, 1], F32)
                    nc.sync.dma_start(out=bar_in[:],
                                      in_=sum_stripe[:, gb_idx:gb_idx + 1])
                    if not cfg.skip_collective:
                        nc.gpsimd.collective_compute(
                            "AllReduce", mybir.AluOpType.add,
                            replica_groups=[list(range(NCORES))],
                            ins=[bar_in.opt()], outs=[bar_out.opt()],
                        )

            # ---- stats + allreduce ------------------------------------------
            st2 = singles.tile([128, 2], F32)
            nc.vector.tensor_reduce(out=st2[:, 0:1], in_=sum_stripe[:],
                                    axis=mybir.AxisListType.X, op=mybir.AluOpType.add)
            nc.vector.tensor_reduce(out=st2[:, 1:2], in_=sq_stripe[:],
                                    axis=mybir.AxisListType.X, op=mybir.AluOpType.add)
            cc_in = dram.tile([128, 2], F32)
            cc_out = dram.tile([128, 2], F32)
            nc.gpsimd.dma_start(out=cc_in[:], in_=st2[:])
            if cfg.skip_collective:
                nc.gpsimd.dma_start(out=cc_out[:], in_=cc_in[:])
            else:
                nc.gpsimd.collective_compute(
                    "AllReduce", mybir.AluOpType.add,
                    replica_groups=[list(range(NCORES))],
                    ins=[cc_in.opt()], outs=[cc_out.opt()],
                )
            gst = singles.tile([128, 2], F32)
            nc.gpsimd.dma_start(out=gst[:], in_=cc_out[:])

            inv_e = 1.0 / cfg.E_TOTAL
            mean_t = singles.tile([128, 1], F32)
            nc.scalar.mul(out=mean_t[:], in_=gst[:, 0:1], mul=inv_e)
            msq_t = singles.tile([128, 1], F32)
            nc.scalar.mul(out=msq_t[:], in_=gst[:, 1:2], mul=inv_e)
            var_t = singles.tile([128, 1], F32)
            nc.vector.tensor_tensor(out=var_t[:], in0=mean_t[:], in1=mean_t[:],
                                    op=mybir.AluOpType.mult)
            nc.vector.tensor_tensor(out=var_t[:], in0=msq_t[:], in1=var_t[:],
                                    op=mybir.AluOpType.subtract)
            eps_t = singles.tile([128, 1], F32)
            nc.vector.memset(eps_t[:], BN_EPS)
            sd_t = singles.tile([128, 1], F32)
            nc.scalar.activation(out=sd_t[:], in_=var_t[:],
                                 func=mybir.ActivationFunctionType.Sqrt,
                                 bias=eps_t[:], scale=1.0)
            rstd_t = singles.tile([128, 1], F32)
            nc.vector.reciprocal(out=rstd_t[:], in_=sd_t[:])
            s_t = singles.tile([128, 1], F32)
            nc.vector.tensor_tensor(out=s_t[:], in0=rstd_t[:], in1=gb_t[:, 0:1],
                                    op=mybir.AluOpType.mult)
            t_t = singles.tile([128, 1], F32)
            nc.vector.tensor_tensor(out=t_t[:], in0=s_t[:], in1=mean_t[:],
                                    op=mybir.AluOpType.mult)
            nc.vector.tensor_tensor(out=t_t[:], in0=gb_t[:, 1:2], in1=t_t[:],
                                    op=mybir.AluOpType.subtract)

            # ---- phase 2: affine + store (feature-major; host transposes) ---
            CH2 = next(c for c in (1024, 512)
                       if cfg.EP % c == 0)
            for k in range(cfg.EP // CH2):
                e0 = k * CH2
                u = ph2.tile([128, CH2], BF16, tag="u")
                if k % 3 == 0:
                    nc.scalar.activation(out=u[:], in_=h3res[:, e0:e0 + CH2],
                                         func=IDENT, bias=t_t[:], scale=s_t[:])
                else:
                    nc.vector.tensor_scalar(out=u[:], in0=h3res[:, e0:e0 + CH2],
                                            scalar1=s_t[:], scalar2=t_t[:],
                                            op0=mybir.AluOpType.mult,
                                            op1=mybir.AluOpType.add)
                nc.sync.dma_start(out=out[:, e0:e0 + CH2], in_=u[:])
    nc.compile()
    return nc


def get_program(cfg):
    key = (cfg.ES, cfg.H, cfg.C, cfg.B, cfg.TN, cfg.E_TOTAL, cfg.skip_collective,
           cfg.NQ, cfg.GSD_BUFS)
    if key not in _PROG_CACHE:
        _PROG_CACHE[key] = build_program(cfg)
    return _PROG_CACHE[key]


def _wrap_idx(flat):
    """int16 flat idxs [n] -> wrapped [128, n/16] layout (i at [i%16, i//16], x8)."""
    w = flat.reshape(-1, 16).T.astype(np.int16)      # [16, n/16]
    return np.tile(w, (8, 1))                        # [128, n/16]


def host_prep(node_feats, edge_feats, src, dst, W1, b1, W2, b2, W3, b3, gamma, beta,
              cfg=None):
    cfg = cfg or CFG
    nfb = np.asarray(node_feats, np.float32).astype(ml_dtypes.bfloat16)
    efb = np.asarray(edge_feats, np.float32).astype(ml_dtypes.bfloat16)
    src = np.asarray(src)
    dst = np.asarray(dst)
    W1 = np.asarray(W1, np.float32)

    w1e = W1[:ED].astype(ml_dtypes.bfloat16)
    w1m = W1[ED:].astype(ml_dtypes.bfloat16)
    w2b = np.asarray(W2, np.float32).astype(ml_dtypes.bfloat16)
    w3b = np.asarray(W3, np.float32).astype(ml_dtypes.bfloat16)
    bias = np.stack([np.asarray(b1, np.float32),
                     np.asarray(b2, np.float32),
                     np.asarray(b3, np.float32)], axis=1)          # [128, 3]
    gbv = np.stack([np.asarray(gamma, np.float32),
                    np.asarray(beta, np.float32)], axis=1)         # [128, 2]

    in_maps = []
    for c in range(NCORES):
        base = c * cfg.ES
        tabs, sws = [], []
        for h in range(cfg.H):
            lo = base + h * cfg.HV
            s_h = src[lo:lo + cfg.HV]
            d_h = dst[lo:lo + cfg.HV]
            u = np.unique(np.concatenate([s_h, d_h]))
            assert len(u) <= cfg.TN, f"local table overflow: {len(u)} > {cfg.TN}"
            assert len(u) <= 32768, "int16 index overflow"
            tab = np.zeros((cfg.TN, ND), ml_dtypes.bfloat16)
            tab[:len(u)] = nfb[u]
            tabs.append(tab)
            s16 = np.searchsorted(u, s_h).astype(np.int16)
            d16 = np.searchsorted(u, d_h).astype(np.int16)
            pad = cfg.HP - cfg.HV
            if pad:
                s16 = np.concatenate([s16, np.zeros(pad, np.int16)])
                d16 = np.concatenate([d16, np.zeros(pad, np.int16)])
            # one wrapped array per gather call: src block then dst block
            for cl in range(cfg.C):
                lo2 = cl * cfg.NCALL
                blk = np.concatenate([s16[lo2:lo2 + cfg.NCALL],
                                      d16[lo2:lo2 + cfg.NCALL]])
                sws.append(_wrap_idx(blk))
        srcw = np.concatenate(sws, axis=1)
        # edge feats, transposed + per-half padding
        eftc = np.zeros((ED, cfg.EP), ml_dtypes.bfloat16)
        for h in range(cfg.H):
            lo = base + h * cfg.HV
            eftc[:, h * cfg.HP:h * cfg.HP + cfg.HV] = efb[lo:lo + cfg.HV].T
        im = {"srcw": srcw, "eft": eftc,
              "w1e": w1e, "w1m": w1m, "w2": w2b, "w3": w3b,
              "bias": bias, "gb": gbv}
        for h in range(cfg.H):
            im[f"tab{h}"] = tabs[h]
        in_maps.append(im)
    return in_maps


def assemble_output(results, cfg=None):
    cfg = cfg or CFG
    out = np.empty((NCORES * cfg.ES, OUT), np.float32)
    for c in range(NCORES):
        oc = np.asarray(results[c]["out"]).astype(np.float32)   # [128, EP] bf16
        for h in range(cfg.H):
            lo = c * cfg.ES + h * cfg.HV
            out[lo:lo + cfg.HV] = oc[:, h * cfg.HP:h * cfg.HP + cfg.HV].T
    return out


def kernel(**inputs):
    cfg = CFG
    nc = get_program(cfg)
    in_maps = host_prep(**inputs, cfg=cfg)
    res = run_bass_kernel_spmd(nc, in_maps, list(range(NCORES)))
    return assemble_output(res.results, cfg)


# revision 19
# speedup vs baseline: 1.1059x; 1.1059x over previous
"""Trainium2 Bass kernel for nn_CysInteractLayer (GNN message-passing layer).

out = BN(lrelu(lrelu(lrelu([ef | nf[src]+nf[dst]] @ W1 + b1) @ W2 + b2) @ W3 + b3))

Sharding: edges across 8 NeuronCores; node_feats/weights replicated
(as per-half local node tables so dma_gather's int16 indices suffice);
BN batch stats all-reduced across cores on-device.  Final output stays
feature-major bf16 [128, E/8] and is transposed/upcast on host.

Gather strategy: NON-transpose dma_gather calls round-robined across the
4 SWDGE queues (each queue runs on its own Q7 core pair, so 4 calls'
descriptor generation overlaps; the transpose-mode gather is NOT safe to
run concurrently across queues - its XBAR spray streams interleave and
corrupt data).  Gathered rows land edge-major [128, rows, 128]; src+dst
are summed on DVE and transposed to feature-major on the PE (via
identity matmul into PSUM), then the 3-layer MLP runs as before.
"""
import numpy as np
import ml_dtypes

import concourse.bass as bass
import concourse.bacc as bacc
import concourse.tile as tile
from concourse import mybir
from concourse.bass_utils import run_bass_kernel_spmd
from concourse.masks import make_identity

F32 = mybir.dt.float32
BF16 = mybir.dt.bfloat16
I16 = mybir.dt.int16

# problem constants (hardcoded per harness contract)
V, E = 50000, 400000
ND, ED, OUT = 128, 64, 128
IN = ND + ED
NEG_SLOPE = 0.01
BN_EPS = 1e-5

NCORES = 8
ES = E // NCORES            # 50000 edges per core

class Cfg:
    """Geometry of the per-core kernel."""

    def __init__(self, es=ES, halves=2, calls_per_half=10, blocks_per_call=5,
                 tn=32768, e_total=E, skip_collective=False, nqueues=4,
                 gsd_bufs=5):
        self.skip_collective = skip_collective
        self.NQ = nqueues                   # SWDGE queues for gather round-robin
        self.GSD_BUFS = gsd_bufs
        self.ES = es                        # valid edges per core
        self.H = halves
        self.C = calls_per_half
        self.B = blocks_per_call
        self.BLK = 512
        self.TN = tn                        # local table rows (padded)
        self.E_TOTAL = e_total
        self.HP = self.C * self.B * self.BLK   # padded edges per half
        self.EP = self.H * self.HP             # padded edges per core
        self.HV = es // halves                 # valid edges per half
        assert es % halves == 0
        assert self.HP >= self.HV
        self.NB = self.H * self.C * self.B     # total blocks
        self.NCALL = self.B * self.BLK         # edges per gather call
        self.R = self.NCALL // 128             # row-chunks per call (per src/dst)
        self.WCOLS = self.NCALL // 16          # wrapped idx cols per call

    def block_valid(self, b):
        """Valid columns in global block b (rest is padding)."""
        h, lb = divmod(b, self.C * self.B)
        lo = lb * self.BLK
        return int(np.clip(self.HV - lo, 0, self.BLK))


CFG = Cfg()

_PROG_CACHE = {}


def build_program(cfg):
    nc = bacc.Bacc(None, target_bir_lowering=False, num_swdge_queues=4)

    tabs = [nc.declare_dram_parameter(f"tab{h}", [cfg.TN, ND], BF16, isOutput=False)
            for h in range(cfg.H)]
    srcw = nc.declare_dram_parameter("srcw", [128, 2 * cfg.H * cfg.C * cfg.WCOLS], I16, isOutput=False)
    eft = nc.declare_dram_parameter("eft", [ED, cfg.EP], BF16, isOutput=False)
    w1e = nc.declare_dram_parameter("w1e", [ED, OUT], BF16, isOutput=False)
    w1m = nc.declare_dram_parameter("w1m", [ND, OUT], BF16, isOutput=False)
    w2 = nc.declare_dram_parameter("w2", [OUT, OUT], BF16, isOutput=False)
    w3 = nc.declare_dram_parameter("w3", [OUT, OUT], BF16, isOutput=False)
    bias = nc.declare_dram_parameter("bias", [128, 3], F32, isOutput=False)   # b1|b2|b3
    gb = nc.declare_dram_parameter("gb", [128, 2], F32, isOutput=False)       # gamma|beta
    out = nc.declare_dram_parameter("out", [128, cfg.EP], BF16, isOutput=True)

    LR = mybir.ActivationFunctionType.Lrelu
    IDENT = mybir.ActivationFunctionType.Identity
    BLK = cfg.BLK
    CPB = BLK // 128          # row-chunks per block (4)

    with tile.TileContext(nc) as tc:
        with (
            tc.tile_pool(name="singles", bufs=1) as singles,
            tc.tile_pool(name="hres", bufs=1) as hres,
            tc.tile_pool(name="eftp", bufs=3) as eftp,
            tc.tile_pool(name="gsdp", bufs=cfg.GSD_BUFS) as gsdp,
            tc.tile_pool(name="sums", bufs=3) as sums,
            tc.tile_pool(name="gtp", bufs=3) as gtp,
            tc.tile_pool(name="work", bufs=3) as work,
            tc.tile_pool(name="ph2", bufs=4) as ph2,
            tc.tile_pool(name="zps", bufs=4, space="PSUM") as zps,
            tc.tile_pool(name="zpt", bufs=2, space="PSUM") as zpt,
            tc.tile_pool(name="dram", bufs=1, space="DRAM") as dram,
        ):
            # ---- static state ------------------------------------------------
            w1e_t = singles.tile([ED, OUT], BF16)
            nc.sync.dma_start(out=w1e_t[:], in_=w1e[:, :])
            w1m_t = singles.tile([ND, OUT], BF16)
            nc.sync.dma_start(out=w1m_t[:], in_=w1m[:, :])
            w2_t = singles.tile([OUT, OUT], BF16)
            nc.sync.dma_start(out=w2_t[:], in_=w2[:, :])
            w3_t = singles.tile([OUT, OUT], BF16)
            nc.sync.dma_start(out=w3_t[:], in_=w3[:, :])
            bias_t = singles.tile([128, 3], F32)
            nc.sync.dma_start(out=bias_t[:], in_=bias[:, :])
            gb_t = singles.tile([128, 2], F32)
            nc.sync.dma_start(out=gb_t[:], in_=gb[:, :])
            ident_t = singles.tile([128, 128], BF16)
            make_identity(nc, ident_t[:])
            # all wrapped gather indices, preloaded once (decouples the
            # gather pipeline from the per-call eft loads on the sync queue)
            sw_all = singles.tile([128, 2 * cfg.H * cfg.C * cfg.WCOLS], I16)
            nc.sync.dma_start(out=sw_all[:], in_=srcw[:, :])

            h3res = hres.tile([128, cfg.EP], BF16)
            sum_stripe = singles.tile([128, cfg.NB], F32)
            sq_stripe = singles.tile([128, cfg.NB], F32)
            # fully-padded blocks are skipped in the loop below; zero the
            # stripes so their (never-written) columns don't poison the stats
            nc.vector.memset(sum_stripe[:], 0.0)
            nc.vector.memset(sq_stripe[:], 0.0)

            # ---- phase 1: gather + sum + transpose + MLP + stats -------------
            for ci in range(cfg.H * cfg.C):
                h = ci // cfg.C
                call_e0 = ci * cfg.NCALL
                eft_t = eftp.tile([ED, cfg.NCALL], BF16, tag="eft")
                nc.sync.dma_start(out=eft_t[:], in_=eft[:, call_e0:call_e0 + cfg.NCALL])

                # non-transpose gather: src rows land in chunks [0,R), dst rows
                # in chunks [R,2R); edge (c*128+p) of this call -> [p, c, :]
                gsd = gsdp.tile([128, 2 * cfg.R, ND], BF16, tag="gsd")
                nc.gpsimd.dma_gather(
                    out_ap=gsd[:, :, :], in_ap=tabs[h][:, :],
                    idxs_ap=sw_all[:, ci * 2 * cfg.WCOLS:(ci + 1) * 2 * cfg.WCOLS],
                    num_idxs=2 * cfg.NCALL, num_idxs_reg=2 * cfg.NCALL,
                    elem_size=ND, transpose=False, single_packet=False,
                    queue_num=ci % cfg.NQ,
                )

                for b in range(cfg.B):
                    gb_idx = ci * cfg.B + b
                    vb = cfg.block_valid(gb_idx)
                    if vb == 0:
                        continue
                    co = b * BLK              # col offset within call
                    e0 = call_e0 + co         # global (padded) edge offset
                    c0 = b * CPB              # first row-chunk of this block

                    # m = nf[src] + nf[dst], edge-major [128, BLK]
                    s_t = sums.tile([128, CPB, 128], BF16, tag="s")
                    nc.vector.tensor_add(
                        out=s_t[:, :, :],
                        in0=gsd[:, c0:c0 + CPB, :],
                        in1=gsd[:, cfg.R + c0:cfg.R + c0 + CPB, :])
                    # transpose to feature-major via PE identity matmuls
                    zt = zpt.tile([128, BLK], BF16, tag="zt")
                    for c in range(CPB):
                        nc.tensor.transpose(zt[:, c * 128:(c + 1) * 128],
                                            s_t[:, c, :], ident_t[:])
                    gt = gtp.tile([128, BLK], BF16, tag="gt")
                    nc.vector.tensor_copy(out=gt[:], in_=zt[:])

                    zp = zps.tile([128, BLK], F32, tag="z")
                    nc.tensor.matmul(zp[:, :], lhsT=w1m_t[:], rhs=gt[:],
                                     start=True, stop=False)
                    nc.tensor.matmul(zp[:, :], lhsT=w1e_t[:], rhs=eft_t[:, co:co + BLK],
                                     start=False, stop=True)
                    h1 = work.tile([128, BLK], BF16, tag="h1")
                    nc.scalar.activation(out=h1[:], in_=zp[:], func=LR,
                                         bias=bias_t[:, 0:1], scale=1.0, alpha=NEG_SLOPE)
                    zp2 = zps.tile([128, BLK], F32, tag="z")
                    nc.tensor.matmul(zp2[:, :], lhsT=w2_t[:], rhs=h1[:], start=True, stop=True)
                    h2 = work.tile([128, BLK], BF16, tag="h2")
                    nc.scalar.activation(out=h2[:], in_=zp2[:], func=LR,
                                         bias=bias_t[:, 1:2], scale=1.0, alpha=NEG_SLOPE)
                    zp3 = zps.tile([128, BLK], F32, tag="z")
                    nc.tensor.matmul(zp3[:, :], lhsT=w3_t[:], rhs=h2[:], start=True, stop=True)
                    h3 = h3res[:, e0:e0 + BLK]
                    nc.scalar.activation(out=h3[:, :vb], in_=zp3[:, :vb], func=LR,
                                         bias=bias_t[:, 2:3], scale=1.0, alpha=NEG_SLOPE,
                                         accum_out=sum_stripe[:, gb_idx:gb_idx + 1])
                    if vb < BLK:
                        nc.vector.memset(h3[:, vb:], 0.0)
                    sq_scr = work.tile([128, BLK], BF16, tag="sq")
                    nc.vector.tensor_mul(out=sq_scr[:], in0=h3[:, :], in1=h3[:, :])
                    nc.vector.tensor_reduce(
                        out=sq_stripe[:, gb_idx:gb_idx + 1], in_=sq_scr[:],
                        axis=mybir.AxisListType.X, op=mybir.AluOpType.add)

                if ci == cfg.H * cfg.C - 4:
                    # early barrier: dummy allreduce on the CC cores, hidden
                    # under the remaining gather calls.  Absorbs inter-core
                    # launch stagger so the real stats allreduce at the end
                    # waits only on post-barrier jitter.
                    bar_in = dram.tile([128, 1], F32)
                    bar_out = dram.tile([128, 1], F32)
                    nc.sync.dma_start(out=bar_in[:],
                                      in_=sum_stripe[:, gb_idx:gb_idx + 1])
                    if not cfg.skip_collective:
                        nc.gpsimd.collective_compute(
                            "AllReduce", mybir.AluOpType.add,
                            replica_groups=[list(range(NCORES))],
                            ins=[bar_in.opt()], outs=[bar_out.opt()],
                        )

            # ---- stats + allreduce ------------------------------------------
            st2 = singles.tile([128, 2], F32)
            nc.vector.tensor_reduce(out=st2[:, 0:1], in_=sum_stripe[:],
                                    axis=mybir.AxisListType.X, op=mybir.AluOpType.add)
            nc.vector.tensor_reduce(out=st2[:, 1:2], in_=sq_stripe[:],
                                    axis=mybir.AxisListType.X, op=mybir.AluOpType.add)
            cc_in = dram.tile([128, 2], F32)
            cc_out = dram.tile([128, 2], F32)
            nc.gpsimd.dma_start(out=cc_in[:], in_=st2[:])
            if cfg.skip_collective:
                nc.gpsimd.dma_start(out=cc_out[:], in_=cc_in[:])
            else:
                nc.gpsimd.collective_compute(
                    "AllReduce", mybir.AluOpType.add,
                    replica_groups=[list(range(NCORES))],
                    ins=[cc_in.opt()], outs=[cc_out.opt()],
                )
            gst = singles.tile([128, 2], F32)
            nc.gpsimd.dma_start(out=gst[:], in_=cc_out[:])

            inv_e = 1.0 / cfg.E_TOTAL
            mean_t = singles.tile([128, 1], F32)
            nc.scalar.mul(out=mean_t[:], in_=gst[:, 0:1], mul=inv_e)
            msq_t = singles.tile([128, 1], F32)
            nc.scalar.mul(out=msq_t[:], in_=gst[:, 1:2], mul=inv_e)
            var_t = singles.tile([128, 1], F32)
            nc.vector.tensor_tensor(out=var_t[:], in0=mean_t[:], in1=mean_t[:],
                                    op=mybir.AluOpType.mult)
            nc.vector.tensor_tensor(out=var_t[:], in0=msq_t[:], in1=var_t[:],
                                    op=mybir.AluOpType.subtract)
            eps_t = singles.tile([128, 1], F32)
            nc.vector.memset(eps_t[:], BN_EPS)
            sd_t = singles.tile([128, 1], F32)
            nc.scalar.activation(out=sd_t[:], in_=var_t[:],
                                 func=mybir.ActivationFunctionType.Sqrt,
                                 bias=eps_t[:], scale=1.0)
            rstd_t = singles.tile([128, 1], F32)
            nc.vector.reciprocal(out=rstd_t[:], in_=sd_t[:])
            s_t = singles.tile([128, 1], F32)
            nc.vector.tensor_tensor(out=s_t[:], in0=rstd_t[:], in1=gb_t[:, 0:1],
                                    op=mybir.AluOpType.mult)
            t_t = singles.tile([128, 1], F32)
            nc.vector.tensor_tensor(out=t_t[:], in0=s_t[:], in1=mean_t[:],
                                    op=mybir.AluOpType.mult)
            nc.vector.tensor_tensor(out=t_t[:], in0=gb_t[:, 1:2], in1=t_t[:],
                                    op=mybir.AluOpType.subtract)

            # ---- phase 2: affine + store (feature-major; host transposes) ---
            CH2 = next(c for c in (1024, 512)
                       if cfg.EP % c == 0)
            for k in range(cfg.EP // CH2):
                e0 = k * CH2
                u = ph2.tile([128, CH2], BF16, tag="u")
                if k % 3 == 0:
                    nc.scalar.activation(out=u[:], in_=h3res[:, e0:e0 + CH2],
                                         func=IDENT, bias=t_t[:], scale=s_t[:])
                else:
                    nc.vector.tensor_scalar(out=u[:], in0=h3res[:, e0:e0 + CH2],
                                            scalar1=s_t[:], scalar2=t_t[:],
                                            op0=mybir.AluOpType.mult,
                                            op1=mybir.AluOpType.add)
                nc.sync.dma_start(out=out[:, e0:e0 + CH2], in_=u[:])
    nc.compile()
    return nc


def get_program(cfg):
    key = (cfg.ES, cfg.H, cfg.C, cfg.B, cfg.TN, cfg.E_TOTAL, cfg.skip_collective,
           cfg.NQ, cfg.GSD_BUFS)
    if key not in _PROG_CACHE:
        _PROG_CACHE[key] = build_program(cfg)
    return _PROG_CACHE[key]


def _wrap_idx(flat):
    """int16 flat idxs [n] -> wrapped [128, n/16] layout (i at [i%16, i//16], x8)."""
    w = flat.reshape(-1, 16).T.astype(np.int16)      # [16, n/16]
    return np.tile(w, (8, 1))                        # [128, n/16]


def host_prep(node_feats, edge_feats, src, dst, W1, b1, W2, b2, W3, b3, gamma, beta,
              cfg=None):
    cfg = cfg or CFG
    nfb = np.asarray(node_feats, np.float32).astype(ml_dtypes.bfloat16)
    efb = np.asarray(edge_feats, np.float32).astype(ml_dtypes.bfloat16)
    src = np.asarray(src)
    dst = np.asarray(dst)
    W1 = np.asarray(W1, np.float32)

    w1e = W1[:ED].astype(ml_dtypes.bfloat16)
    w1m = W1[ED:].astype(ml_dtypes.bfloat16)
    w2b = np.asarray(W2, np.float32).astype(ml_dtypes.bfloat16)
    w3b = np.asarray(W3, np.float32).astype(ml_dtypes.bfloat16)
    bias = np.stack([np.asarray(b1, np.float32),
                     np.asarray(b2, np.float32),
                     np.asarray(b3, np.float32)], axis=1)          # [128, 3]
    gbv = np.stack([np.asarray(gamma, np.float32),
                    np.asarray(beta, np.float32)], axis=1)         # [128, 2]

    in_maps = []
    for c in range(NCORES):
        base = c * cfg.ES
        tabs, sws = [], []
        for h in range(cfg.H):
            lo = base + h * cfg.HV
            s_h = src[lo:lo + cfg.HV]
            d_h = dst[lo:lo + cfg.HV]
            u = np.unique(np.concatenate([s_h, d_h]))
            assert len(u) <= cfg.TN, f"local table overflow: {len(u)} > {cfg.TN}"
            assert len(u) <= 32768, "int16 index overflow"
            tab = np.zeros((cfg.TN, ND), ml_dtypes.bfloat16)
            tab[:len(u)] = nfb[u]
            tabs.append(tab)
            s16 = np.searchsorted(u, s_h).astype(np.int16)
            d16 = np.searchsorted(u, d_h).astype(np.int16)
            pad = cfg.HP - cfg.HV
            if pad:
                s16 = np.concatenate([s16, np.zeros(pad, np.int16)])
                d16 = np.concatenate([d16, np.zeros(pad, np.int16)])
            # one wrapped array per gather call: src block then dst block
            for cl in range(cfg.C):
                lo2 = cl * cfg.NCALL
                blk = np.concatenate([s16[lo2:lo2 + cfg.NCALL],
                                      d16[lo2:lo2 + cfg.NCALL]])
                sws.append(_wrap_idx(blk))
        srcw = np.concatenate(sws, axis=1)
        # edge feats, transposed + per-half padding
        eftc = np.zeros((ED, cfg.EP), ml_dtypes.bfloat16)
        for h in range(cfg.H):
            lo = base + h * cfg.HV
            eftc[:, h * cfg.HP:h * cfg.HP + cfg.HV] = efb[lo:lo + cfg.HV].T
        im = {"srcw": srcw, "eft": eftc,
              "w1e": w1e, "w1m": w1m, "w2": w2b, "w3": w3b,
              "bias": bias, "gb": gbv}
        for h in range(cfg.H):
            im[f"tab{h}"] = tabs[h]
        in_maps.append(im)
    return in_maps


def assemble_output(results, cfg=None):
    cfg = cfg or CFG
    out = np.empty((NCORES * cfg.ES, OUT), np.float32)
    for c in range(NCORES):
        oc = np.asarray(results[c]["out"]).astype(np.float32)   # [128, EP] bf16
        for h in range(cfg.H):
            lo = c * cfg.ES + h * cfg.HV
            out[lo:lo + cfg.HV] = oc[:, h * cfg.HP:h * cfg.HP + cfg.HV].T
    return out


def kernel(**inputs):
    cfg = CFG
    nc = get_program(cfg)
    in_maps = host_prep(**inputs, cfg=cfg)
    res = run_bass_kernel_spmd(nc, in_maps, list(range(NCORES)))
    return assemble_output(res.results, cfg)
